# revision 6
# baseline (speedup 1.0000x reference)
"""ButterflyLinear Trainium2 kernel (v2).

Math: out[b, s, i] = (sum_o x[b, s, o] * W[o, i]) * mask[s, i], with
mask[s, i] = 1 iff 4s <= i < 4s+4 (stride-4 band). Only the 4-wide band
is computed; the host scatters it into the zero-filled full output.

Sharding (8 cores): core t owns s-block t = s in [128t, 128t+128) for
all 16 batches (output columns [512t, 512t+512); no inter-core comm).

Per-core structure: s_rel = 32h + r (h: 4 PSUM banks, r in [0,32)),
o = 128c + p (c: 8 contraction chunks, p: partition). For each (c, h)
ONE matmul: stationary = W[128c:128c+128, 512t+128h : +128] (128x128),
moving = x pack [p, (b, r)] (128 x 512, fp16), accumulated over c into
PSUM bank h: bank_h[w, 32b + r] = y[b, 128t+32h+r, 512t+128h+w].
32 matmuls of N=512 instead of 128 of N=128.

Schedule: a dummy-matmul warmup stream keeps TensorE busy from t~0 so
the HAM clock gate reaches 2.4 GHz before real matmuls; W arrives as
one 1MB DMA, x as 8 chunk DMAs (512KB, per-partition contiguous, FIFO
on the Sync HWDGE ring) so the DMA stream saturates HBM; each bank is
drained (Vector/Scalar alternating, fp32->fp16) right after its last
matmul and shipped out in 2 DMAs.

Host extracts the band: band[b, 128t+32h+r, j] = out_t[4r+j, h, 32b+r].
"""

import os
import sys
from contextlib import ExitStack

import numpy as np

if "/opt/trn_rl_repo" not in sys.path:
    sys.path.insert(0, "/opt/trn_rl_repo")

import concourse.bass as bass  # noqa: E402
import concourse.tile as tile  # noqa: E402
from concourse import bacc, mybir  # noqa: E402
from concourse.bass_utils import run_bass_kernel_spmd  # noqa: E402

B = 16  # batch
NT = 8  # s-blocks == cores
SB = 128  # s rows per block
NCH = 8  # o chunks
KC = 128  # o rows per chunk
NH = 4  # PSUM banks / 128-col W windows per block
R = 32  # s rows per window
U = B * R  # moving free size = 512

NWARM = int(os.environ.get("BFK_NWARM", "16"))  # warmup dummy matmuls
DUALQ = os.environ.get("BFK_DUALQ", "0") == "1"  # x chunks on 2 HWDGE rings

MM_DT = mybir.dt.float16
F32 = mybir.dt.float32

_STATE: dict = {}


def _build():
    if "nc" in _STATE:
        return _STATE["nc"]

    nc = bacc.Bacc("TRN2", target_bir_lowering=False, debug=False, num_devices=NT)
    # Partition-major DRAM layouts: one descriptor row per partition.
    xt = nc.dram_tensor("xt", [KC, NCH, NH, B, R], MM_DT, kind="ExternalInput").ap()
    wt = nc.dram_tensor("wt", [KC, NCH, NH, KC], MM_DT, kind="ExternalInput").ap()
    out = nc.dram_tensor("out", [KC, NH, U], mybir.dt.float16, kind="ExternalOutput").ap()

    with tile.TileContext(nc) as tc, ExitStack() as ctx:
        wp = ctx.enter_context(tc.tile_pool(name="w", bufs=1))
        xp = ctx.enter_context(tc.tile_pool(name="x", bufs=1))
        pp = ctx.enter_context(tc.tile_pool(name="ps", bufs=1, space="PSUM"))
        op = ctx.enter_context(tc.tile_pool(name="o", bufs=3))

        # Warmup scratch: zeros SBUF tile + scratch PSUM bank. The dummy
        # matmuls keep the PE HAM activity window busy from kernel start
        # so the real matmuls run at 2.4 GHz instead of 1.2.
        wsc = op.tile([KC, U], MM_DT, tag="warm")
        nc.gpsimd.memset(wsc[:], 0.0)
        psw = pp.tile([SB, U], F32, tag="psw")

        # Input DMAs: W first (covers every chunk), then x chunks in
        # consumption order; FIFO on the ring keeps the stream dense.
        w_all = wp.tile([KC, NCH, NH, KC], MM_DT, tag="w")
        nc.sync.dma_start(out=w_all[:], in_=wt[:])
        x_t = []
        for c in range(NCH):
            xc = xp.tile([KC, NH, B, R], MM_DT, tag=f"x{c}")
            eng = nc.scalar if (DUALQ and c % 2 == 1) else nc.sync
            eng.dma_start(out=xc[:], in_=xt[:, c])
            x_t.append(xc)

        for i in range(NWARM):
            nc.tensor.matmul(psw[:], wsc[:, :KC], wsc[:], start=True, stop=True)

        ps = [
            pp.tile([SB, U], F32, tag=f"ps{h}", name=f"ps_{h}") for h in range(NH)
        ]
        for c in range(NCH):
            for h in range(NH):
                nc.tensor.matmul(
                    ps[h][:],
                    w_all[:, c, h, :],
                    x_t[c][:, h],
                    start=(c == 0),
                    stop=(c == NCH - 1),
                )

        # Drain banks as their accumulation chains finish; alternate
        # Vector/Scalar so two banks drain at once.
        ot01 = op.tile([KC, 2, U], mybir.dt.float16, tag="ot01")
        ot23 = op.tile([KC, 2, U], mybir.dt.float16, tag="ot23")
        nc.vector.tensor_copy(ot01[:, 0], ps[0][:])
        nc.scalar.copy(ot01[:, 1], ps[1][:])
        nc.vector.tensor_copy(ot23[:, 0], ps[2][:])
        nc.scalar.copy(ot23[:, 1], ps[3][:])
        nc.sync.dma_start(out=out[:, 0:2], in_=ot01[:])
        nc.sync.dma_start(out=out[:, 2:4], in_=ot23[:])

    nc.compile()
    _STATE["nc"] = nc
    return nc


def _shard(x, W):
    np_dt = mybir.dt.np(MM_DT)
    x = np.ascontiguousarray(np.asarray(x, dtype=np.float32)).astype(np_dt)
    W = np.ascontiguousarray(np.asarray(W, dtype=np.float32)).astype(np_dt)
    # xt[t][p, c, h, b, r] = x[b, 128t + 32h + r, 128c + p]
    xr = x.reshape(B, NT, NH, R, NCH, KC)  # [b, t, h, r, c, p]
    xts = np.ascontiguousarray(np.transpose(xr, (1, 5, 4, 2, 0, 3)))
    # wt[t][p, c, h, w] = W[128c + p, 512t + 128h + w]
    wr = W.reshape(NCH, KC, NT, NH, KC)  # [c, p, t, h, w]
    wts = np.ascontiguousarray(np.transpose(wr, (2, 1, 0, 3, 4)))
    return [{"xt": xts[t], "wt": wts[t]} for t in range(NT)]


def kernel(x, W, _trace=False, _trace_kwargs=None):
    nc = _build()
    in_maps = _shard(x, W)
    res = run_bass_kernel_spmd(
        nc,
        in_maps,
        list(range(NT)),
        trace=_trace,
        **(_trace_kwargs or {}),
    )
    _STATE["last_run"] = res
    band = np.empty((B, NT * SB, 4), dtype=np.float32)
    for t in range(NT):
        blk = res.results[t]["out"].astype(np.float32)  # (128, NH, U)
        v = blk.reshape(R, 4, NH, B, R)  # [r', j, h, b, r]; band at r' == r
        band[:, t * SB : (t + 1) * SB, :] = np.einsum("rjhbr->bhrj", v).reshape(
            B, SB, 4
        )
    s_idx = np.arange(NT * SB)
    y = np.zeros((B, NT * SB, NT * SB, 4), dtype=np.float32)
    y[:, s_idx, s_idx, :] = band
    return y.reshape(B, NT * SB, NT * SB * 4)


# revision 7
# speedup vs baseline: 1.2498x; 1.2498x over previous
"""ButterflyLinear Trainium2 kernel (v3).

Math: out[b, s, i] = (sum_o x[b, s, o] * W[o, i]) * mask[s, i], with
mask[s, i] = 1 iff 4s <= i < 4s+4 (stride-4 band). Only the 4-wide band
is computed; the host scatters it into the zero-filled full output.

Sharding (8 cores): core t owns s-block t = s in [128t, 128t+128) for
all 16 batches (output columns [512t, 512t+512); no inter-core comm).

Per-core structure: s_rel = 32h + r (h: 4 PSUM banks, r in [0,32)),
o = 128c + p (c: 8 contraction chunks, p: partition). For each (c, h)
ONE matmul: stationary = W[128c:128c+128, 512t+128h : +128] (128x128
fp16), moving = x pack [p, (b, r)] (128 x 512, fp8 e3m4), accumulated
over c in fp32 PSUM bank h: bank_h[w, 32b+r] = y[b, 128t+32h+r,
512t+128h+w]. 32 matmuls of N=512.

x travels as fp8 e3m4 (4 mantissa bits): halves the dominant DMA
stream (4MB -> 2MB per core); band rel err ~1.5e-2 vs the 2e-2 gate
(quantization happens on host, so the error is deterministic). W stays
fp16 (mixed-dtype matmul; fp8 W on top would put err at ~2e-2).

Schedule: warmup dummy matmuls keep TensorE busy from t~0 so the HAM
clock gate reaches 2.4 GHz before real matmuls; W is interleaved into
the DMA stream in chunk order (W01, x0, x1, W23, x2, ...) on the Sync
HWDGE ring so chunk-0 matmuls start as early as possible; the last x
chunk is split in half so its matmuls/copies overlap the stream tail;
banks drain Vector/Scalar alternating into fp16 and ship in 2 DMAs.

Host extracts the band: band[b, 128t+32h+r, j] = out_t[4r+j, h, 32b+r].
"""

import os
import sys
from contextlib import ExitStack

import numpy as np

if "/opt/trn_rl_repo" not in sys.path:
    sys.path.insert(0, "/opt/trn_rl_repo")

import concourse.bass as bass  # noqa: E402
import concourse.tile as tile  # noqa: E402
from concourse import bacc, mybir  # noqa: E402
from concourse.bass_utils import run_bass_kernel_spmd  # noqa: E402

B = 16  # batch
NT = 8  # s-blocks == cores
SB = 128  # s rows per block
NCH = 8  # o chunks
KC = 128  # o rows per chunk
NH = 4  # PSUM banks / 128-col W windows per block
R = 32  # s rows per window
U = B * R  # moving free size = 512

NWARM = int(os.environ.get("BFK_NWARM", "7"))  # warmup dummy matmuls
_DT = {"f8e3": mybir.dt.float8e3, "f8e4": mybir.dt.float8e4, "f16": mybir.dt.float16}
X_DT = _DT[os.environ.get("BFK_XDT", "f8e3")]
W_DT = _DT[os.environ.get("BFK_WDT", "f16")]
F16 = mybir.dt.float16
F32 = mybir.dt.float32

_STATE: dict = {}


def _build():
    if "nc" in _STATE:
        return _STATE["nc"]

    nc = bacc.Bacc("TRN2", target_bir_lowering=False, debug=False, num_devices=NT)
    # Partition-major DRAM layouts: one descriptor row per partition.
    xt = nc.dram_tensor("xt", [KC, NCH, NH, B, R], X_DT, kind="ExternalInput").ap()
    wt = nc.dram_tensor("wt", [KC, NCH, NH, KC], W_DT, kind="ExternalInput").ap()
    out = nc.dram_tensor("out", [KC, NH, U], F16, kind="ExternalOutput").ap()

    with tile.TileContext(nc) as tc, ExitStack() as ctx:
        wp = ctx.enter_context(tc.tile_pool(name="w", bufs=1))
        xp = ctx.enter_context(tc.tile_pool(name="x", bufs=1))
        pp = ctx.enter_context(tc.tile_pool(name="ps", bufs=1, space="PSUM"))
        op = ctx.enter_context(tc.tile_pool(name="o", bufs=1))

        # Warmup scratch: zeroed SBUF tile + scratch PSUM bank. The dummy
        # matmuls keep the PE HAM activity window busy from kernel start
        # so the real matmuls run at 2.4 GHz instead of 1.2.
        wsc = op.tile([KC, U], F16, tag="warm")
        nc.gpsimd.memset(wsc[:], 0.0)
        psw = pp.tile([SB, U], F32, tag="psw")

        # DMA stream in consumption order: W chunk-pair just ahead of the
        # x chunks that need it; FIFO on the Sync ring keeps it dense.
        w_all = wp.tile([KC, NCH, NH, KC], W_DT, tag="w")
        x_t = []
        for c in range(NCH):
            xc = xp.tile(
                [KC, NH, B, R], X_DT, tag=f"x{c}", name=f"x_{c}"
            )
            x_t.append(xc)
        xl = xp.tile([KC, 2, B, R], X_DT, tag="xl")  # last chunk, 2nd half

        for c in range(NCH):
            if c % 2 == 0:
                nc.sync.dma_start(out=w_all[:, c : c + 2], in_=wt[:, c : c + 2])
            if c == NCH - 1:
                nc.sync.dma_start(out=x_t[c][:, 0:2], in_=xt[:, c, 0:2])
                nc.sync.dma_start(out=xl[:], in_=xt[:, c, 2:4])
            else:
                nc.sync.dma_start(out=x_t[c][:], in_=xt[:, c])

        for i in range(NWARM):
            nc.tensor.matmul(psw[:], wsc[:, :KC], wsc[:], start=True, stop=True)

        ps = [
            pp.tile([SB, U], F32, tag=f"ps{h}", name=f"ps_{h}") for h in range(NH)
        ]
        for c in range(NCH):
            for h in range(NH):
                src = (
                    xl[:, h - 2]
                    if (c == NCH - 1 and h >= 2)
                    else x_t[c][:, h]
                )
                nc.tensor.matmul(
                    ps[h][:],
                    w_all[:, c, h, :],
                    src,
                    start=(c == 0),
                    stop=(c == NCH - 1),
                )

        # Drain banks as their accumulation chains finish; alternate
        # Vector/Scalar so two banks drain at once; 2 out DMAs.
        ot01 = op.tile([KC, 2, U], F16, tag="ot01")
        ot23 = op.tile([KC, 2, U], F16, tag="ot23")
        nc.vector.tensor_copy(ot01[:, 0], ps[0][:])
        nc.scalar.copy(ot01[:, 1], ps[1][:])
        nc.sync.dma_start(out=out[:, 0:2], in_=ot01[:])
        nc.vector.tensor_copy(ot23[:, 0], ps[2][:])
        nc.scalar.copy(ot23[:, 1], ps[3][:])
        nc.sync.dma_start(out=out[:, 2:4], in_=ot23[:])

    nc.compile()
    _STATE["nc"] = nc
    return nc


def _shard(x, W):
    x_np = mybir.dt.np(X_DT)
    w_np = mybir.dt.np(W_DT)
    x = np.ascontiguousarray(np.asarray(x, dtype=np.float32)).astype(x_np)
    W = np.ascontiguousarray(np.asarray(W, dtype=np.float32)).astype(w_np)
    # xt[t][p, c, h, b, r] = x[b, 128t + 32h + r, 128c + p]
    xr = x.reshape(B, NT, NH, R, NCH, KC)  # [b, t, h, r, c, p]
    xts = np.ascontiguousarray(np.transpose(xr, (1, 5, 4, 2, 0, 3)))
    # wt[t][p, c, h, w] = W[128c + p, 512t + 128h + w]
    wr = W.reshape(NCH, KC, NT, NH, KC)  # [c, p, t, h, w]
    wts = np.ascontiguousarray(np.transpose(wr, (2, 1, 0, 3, 4)))
    return [{"xt": xts[t], "wt": wts[t]} for t in range(NT)]


def kernel(x, W, _trace=False, _trace_kwargs=None):
    nc = _build()
    in_maps = _shard(x, W)
    res = run_bass_kernel_spmd(
        nc,
        in_maps,
        list(range(NT)),
        trace=_trace,
        **(_trace_kwargs or {}),
    )
    _STATE["last_run"] = res
    band = np.empty((B, NT * SB, 4), dtype=np.float32)
    for t in range(NT):
        blk = res.results[t]["out"].astype(np.float32)  # (128, NH, U)
        v = blk.reshape(R, 4, NH, B, R)  # [r', j, h, b, r]; band at r' == r
        band[:, t * SB : (t + 1) * SB, :] = np.einsum("rjhbr->bhrj", v).reshape(
            B, SB, 4
        )
    s_idx = np.arange(NT * SB)
    y = np.zeros((B, NT * SB, NT * SB, 4), dtype=np.float32)
    y[:, s_idx, s_idx, :] = band
    return y.reshape(B, NT * SB, NT * SB * 4)
